# revision 78
# baseline (speedup 1.0000x reference)
"""Distributed Trainium2 kernel for pre-LN multi-head self-attention.

Reference computation (n=2048, d=1024, 16 heads x 64):
    xn  = LayerNorm(x) * ln_scale + ln_bias
    qkv = xn @ w_qkv ; split -> q,k,v [16, 2048, 64]
    sim = (q @ k^T) * d**-0.5 ; attn = softmax(sim)
    out = concat_heads(attn @ v) @ w_out + b_out

Sharding: 2 heads per core (tensor parallel). Each core:
  - computes LayerNorm(x) (replicated) and xn^T via PE transposes
    (ln_scale folded into weights on host; ln_bias / b_out folded in as
    rank-1 matmul terms so PSUM evacuations are plain copies)
  - projects its 2 heads' q/k/v; attention in transposed layout (keys
    on partitions); both heads' attn@v run column-tiled concurrently
  - softmax denominators: running DVE partial sums over key chunks,
    then one column-tiled PE matmul does partition-sum + broadcast
  - one AllGather per row-chunk stage, widths tapered
    (512,512,512,384,128) so the tail AllGather is small
  - computes a 128-column slice of the final projection
Host assembles the 8 [128, 2048] outT shards into [2048, 1024].

Scheduling: the prologue is a per-tile software pipeline (LayerNorm of
tile t alongside transposes/evacuation of tile t-1) with merged
single-instruction evacuations so no engine FIFO waits behind another
engine's slow dependency. Stage D is one continuous global key-chunk
stream: sim psum is bf16 (1 bank/chunk) giving a 4-deep ring that
hides cross-engine semaphore latency; attn@v trails at a fixed lag;
norm/AllGather chains and projections slot into the stream without
ever blocking the PE on a collective.
"""

import sys

import ml_dtypes
import numpy as np

for _p in ("/opt/trn_rl_repo", "/root/.axon_site/_ro/trn_rl_repo"):
    if _p not in sys.path:
        sys.path.append(_p)

N = 2048          # sequence length
D = 1024          # model dim
HEADS = 16
DH = 64
NCORES = 8
HL = HEADS // NCORES          # heads per core (2)
HC = HL * DH                  # head cols per core (128)
LN_EPS = 1e-6
SIM_SCALE = float(D) ** -0.5  # reference scales by input dim

P = 128
RT = N // P        # 16 row tiles
DC = D // P        # 8 dim chunks
RC_W = 512         # max row-chunk width
CHUNKS = [(0, 512), (512, 512), (1024, 512), (1536, 384), (1920, 128)]
S = len(CHUNKS)
LAG = 4            # attn@v trails sim by this many global slots

MM_DT = "bf16"
SIM_PSUM_BF16 = False
DEBUG = False

_BUILT = None


def _build():
    """Build the SPMD Bass graph (same graph on all 8 cores)."""
    from contextlib import ExitStack

    import concourse.tile as tile
    from concourse import bacc, mybir
    from concourse.masks import make_identity

    f32 = mybir.dt.float32
    dt_mm = {"f32": f32, "f32r": mybir.dt.float32r,
             "bf16": mybir.dt.bfloat16}[MM_DT]
    dt_sim = dt_mm if SIM_PSUM_BF16 else f32
    AF = mybir.ActivationFunctionType

    nc = bacc.Bacc(None, num_devices=NCORES)

    x_d = nc.declare_dram_parameter("x", [N, D], f32, isOutput=False)
    wq_d = nc.declare_dram_parameter("wq", [D, HC], dt_mm, isOutput=False)
    wk_d = nc.declare_dram_parameter("wk", [D, HC], dt_mm, isOutput=False)
    wv_d = nc.declare_dram_parameter("wv", [D, HC], dt_mm, isOutput=False)
    qb_d = nc.declare_dram_parameter("qb", [HC], dt_mm, isOutput=False)
    kb_d = nc.declare_dram_parameter("kb", [HC], dt_mm, isOutput=False)
    vb_d = nc.declare_dram_parameter("vb", [HC], dt_mm, isOutput=False)
    wo_d = nc.declare_dram_parameter("wo", [D, HC], dt_mm, isOutput=False)
    bo_d = nc.declare_dram_parameter("bo", [HC], f32, isOutput=False)
    out_d = nc.declare_dram_parameter("out", [HC, N], f32, isOutput=True)
    if DEBUG:
        dbg_qt = nc.declare_dram_parameter("dbg_qt", [P, N], f32, isOutput=True)
        dbg_kt = nc.declare_dram_parameter("dbg_kt", [P, N], f32, isOutput=True)
        dbg_v = nc.declare_dram_parameter("dbg_v", [P, RT * HL * DH],
                                          f32, isOutput=True)
        dbg_attn = nc.declare_dram_parameter("dbg_attn", [DH, N], f32,
                                             isOutput=True)

    groups = [list(range(NCORES))]

    with ExitStack() as ctx:
        tc = ctx.enter_context(tile.TileContext(nc))

        dram = ctx.enter_context(tc.tile_pool(name="dram", bufs=1, space="DRAM"))
        ag_in = [dram.tile([P, w], dt_mm, name=f"ag_in{i}")
                 for i, (_, w) in enumerate(CHUNKS)]
        ag_out = [dram.tile([NCORES * P, w], dt_mm, addr_space="Shared",
                            name=f"ag_out{i}") for i, (_, w) in enumerate(CHUNKS)]

        singles = ctx.enter_context(tc.tile_pool(name="singles", bufs=1))
        xp = ctx.enter_context(tc.tile_pool(name="xp", bufs=6))
        xhp = ctx.enter_context(tc.tile_pool(name="xh", bufs=3))
        dump = ctx.enter_context(tc.tile_pool(name="dum", bufs=1, space="PSUM"))

        ident = singles.tile([P, P], dt_mm)
        make_identity(nc, ident)
        warm_rhs = singles.tile([P, RC_W], dt_mm)
        nc.vector.memset(warm_rhs, 0.0)
        eps_t = singles.tile([P, 1], f32)
        nc.vector.memset(eps_t, LN_EPS)
        ones_row = singles.tile([P, RC_W], dt_mm)
        nc.gpsimd.memset(ones_row, 1.0)

        def dummy_mms(n, width=256):
            for _ in range(n):
                dmt = dump.tile([P, RC_W], f32, tag="warm")
                nc.tensor.matmul(dmt[:, 0:width], ident,
                                 warm_rhs[:, 0:width], start=True,
                                 stop=True)

        def qkv_mm(pool, tag, w_sb, bp0, dst, nt, vec_evac):
            """One 512-row projection block: 8 accum MMs + rank-1 bias."""
            pm = pool.tile([P, 512], f32, tag=tag, name=f"pm{bp0}_{nt}")
            for kc in range(DC):
                nc.tensor.matmul(
                    pm, w_sb[:, kc, :],
                    xnT[:, kc, nt * 512:(nt + 1) * 512],
                    start=(kc == 0), stop=False,
                )
            nc.tensor.matmul(
                pm, bias_rows[bp0:bp0 + 1, :], ones_row[bp0:bp0 + 1, :],
                start=False, stop=True,
            )
            with nc.allow_low_precision(reason="qkv evac"):
                if vec_evac:
                    nc.vector.tensor_copy(
                        out=dst[:, nt * 512:(nt + 1) * 512], in_=pm
                    )
                else:
                    nc.scalar.copy(
                        out=dst[:, nt * 512:(nt + 1) * 512], in_=pm
                    )

        def vt_batch(pool, tag, b):
            """v^T -> v (row-major) for 4 row tiles."""
            pt = pool.tile([P, 4, P], f32, tag=tag, name=f"vt{b}")
            for j in range(4):
                rt = b * 4 + j
                nc.tensor.matmul(
                    pt[:, j, :], vT[:, rt * P:(rt + 1) * P], ident,
                    start=True, stop=True,
                )
            with nc.allow_low_precision(reason="v evac"):
                nc.vector.tensor_copy(
                    out=v_sb[:, b * 4:(b + 1) * 4, :, :],
                    in_=pt[:, 0:4, :].rearrange("p j (h d) -> p j h d", h=HL),
                )

        # x tile DMAs first so tile 0's pipeline starts ASAP
        x_tiles = []
        for rt in range(3):
            x_t = xp.tile([P, D], f32, tag="x", name=f"x{rt}")
            nc.sync.dma_start(out=x_t, in_=x_d[rt * P:(rt + 1) * P, :])
            x_tiles.append(x_t)

        # weights / biases (wk first: K projection runs first); biases as
        # single-partition rows for rank-1 accumulation into the qkv psum
        wq_sb = singles.tile([P, DC, HC], dt_mm)
        wk_sb = singles.tile([P, DC, HC], dt_mm)
        wv_sb = singles.tile([P, DC, HC], dt_mm)
        wo_sb = singles.tile([P, DC, HC], dt_mm)
        nc.sync.dma_start(
            out=wk_sb, in_=wk_d[:, :].rearrange("(c p) m -> p c m", p=P)
        )
        nc.sync.dma_start(
            out=wq_sb, in_=wq_d[:, :].rearrange("(c p) m -> p c m", p=P)
        )
        nc.sync.dma_start(
            out=wv_sb, in_=wv_d[:, :].rearrange("(c p) m -> p c m", p=P)
        )
        # bf16 bias rows at stationary-legal partitions: q@0, k@32, v@64
        bias_rows = singles.tile([P, HC], dt_mm)
        bo_r = singles.tile([1, HC], f32)
        bo_row = singles.tile([1, HC], dt_mm)
        for p0, b_d in ((0, qb_d), (32, kb_d), (64, vb_d)):
            nc.sync.dma_start(
                out=bias_rows[p0:p0 + 1, :],
                in_=b_d[:].rearrange("(o m) -> o m", o=1),
            )
        nc.sync.dma_start(out=bo_r, in_=bo_d[:].rearrange("(o m) -> o m", o=1))
        with nc.allow_low_precision(reason="bias row"):
            nc.vector.tensor_copy(out=bo_row, in_=bo_r)

        # long-lived activations
        qT = singles.tile([P, N], dt_mm)        # [2*64 qdims, rows]
        kT = singles.tile([P, N], dt_mm)
        v_sb = singles.tile([P, RT, HL, DH], dt_mm)  # [key%128, kc, h, vdim]
        attn_full = singles.tile([P, N], dt_mm)  # [2 heads x 64 dims, rows]
        xnT = singles.tile([P, DC, N], dt_mm)    # [dim%128, dimchunk, rows]
        vT = singles.tile([P, N], dt_mm)

        # ---- stages A-C: LayerNorm -> xn^T -> q/k/v, per-tile pipeline -----
        with (
            tc.tile_pool(name="stat", bufs=4) as statp,
            tc.tile_pool(name="tp", bufs=2, space="PSUM") as tp,
            tc.tile_pool(name="mmp", bufs=3, space="PSUM") as mmp,
        ):
            # dependency-free burst to cover the first x DMA + LN latency
            dummy_mms(12, width=RC_W)

            def ln_tile(rt):
                if rt < RT - 3:
                    nx = xp.tile([P, D], f32, tag="x", name=f"x{rt + 3}")
                    nc.sync.dma_start(
                        out=nx, in_=x_d[(rt + 3) * P:(rt + 4) * P, :]
                    )
                    x_tiles.append(nx)
                x_t = x_tiles[rt]
                stats = statp.tile([P, 2, 6], f32, tag="st")
                for sg in range(2):
                    nc.vector.bn_stats(
                        out=stats[:, sg, :],
                        in_=x_t[:, sg * 512:(sg + 1) * 512],
                    )
                mv = statp.tile([P, 2], f32, tag=f"mv{rt % 4}")
                nc.vector.bn_aggr(out=mv, in_=stats)
                rstd = statp.tile([P, 1], f32, tag=f"rstd{rt % 4}")
                nc.scalar.activation(
                    out=rstd, in_=mv[:, 1:2], func=AF.Sqrt,
                    bias=eps_t, scale=1.0,
                )
                nc.vector.reciprocal(out=rstd, in_=rstd)
                xh_t = xhp.tile([P, D], dt_mm, tag="xh")
                eng = nc.vector
                eng.tensor_scalar(
                    out=xh_t, in0=x_t,
                    scalar1=mv[:, 0:1], scalar2=rstd,
                    op0=mybir.AluOpType.subtract,
                    op1=mybir.AluOpType.mult,
                )
                x_tiles[rt] = xh_t
                if rt == 3:
                    nc.sync.dma_start(
                        out=wo_sb,
                        in_=wo_d[:, :].rearrange("(c p) m -> p c m", p=P),
                    )

            def transpose_tile(rt):
                xh_t = x_tiles[rt]
                pt = tp.tile([P, DC, P], f32, tag="pt")
                for dc in range(DC):
                    nc.tensor.matmul(
                        pt[:, dc, :],
                        xh_t[:, dc * P:(dc + 1) * P],
                        ident, start=True, stop=True,
                    )
                dst = xnT[:, :, rt * P:(rt + 1) * P]
                with nc.allow_low_precision(reason="transpose evac"):
                    if rt % 4 == 1:
                        nc.vector.tensor_copy(out=dst, in_=pt)
                    else:
                        nc.scalar.copy(out=dst, in_=pt)
                dummy_mms(1)

            for rt in range(RT + 1):
                if rt < RT:
                    ln_tile(rt)
                if rt >= 1:
                    transpose_tile(rt - 1)
                    if (rt - 1) % 4 == 3:
                        # K projection for every block; Q/V only for block 0
                        nt = (rt - 1) // 4
                        qkv_mm(mmp, "pm", wk_sb, 32, kT, nt, False)
                        if nt == 0:
                            qkv_mm(mmp, "pm", wq_sb, 0, qT, 0, False)
                            qkv_mm(mmp, "pm", wv_sb, 64, vT, 0, False)
                        dummy_mms(1)
            vt_batch(mmp, "pm", 0)

        # ---- stage D: attention, one continuous global kc stream -----------
        with (
            tc.tile_pool(name="expp", bufs=2) as expp,
            tc.tile_pool(name="rsum", bufs=2) as rsump,
            tc.tile_pool(name="sp", bufs=4 if SIM_PSUM_BF16 else 2,
                         space="PSUM") as sp,
            tc.tile_pool(name="op", bufs=2, space="PSUM") as op,
            tc.tile_pool(name="prp", bufs=1, space="PSUM") as prp,
            tc.tile_pool(name="agp", bufs=2) as agp,
        ):
            state = {}

            def sim_group(s, kc):
                r0, w = CHUNKS[s]
                st = state[s]
                ps = sp.tile([P, 2, RC_W], dt_sim, tag="ps",
                             name=f"ps{s}_{kc}")
                for h in range(HL):
                    nc.tensor.matmul(
                        ps[:, h, 0:w],
                        kT[h * DH:(h + 1) * DH, kc * P:(kc + 1) * P],
                        qT[h * DH:(h + 1) * DH, r0:r0 + w],
                        start=True, stop=True,
                    )
                nc.scalar.activation(
                    out=st["exp_t"][:, kc, :, 0:w],
                    in_=ps[:, :, 0:w],
                    func=AF.Exp, scale=SIM_SCALE,
                )
                # running denominator partials (per key-in-chunk lane)
                if kc == 1:
                    with nc.allow_low_precision(reason="den partials"):
                        nc.vector.tensor_tensor(
                            out=st["dacc"][:, :, 0:w],
                            in0=st["exp_t"][:, 0, :, 0:w],
                            in1=st["exp_t"][:, 1, :, 0:w],
                            op=mybir.AluOpType.add,
                        )
                elif kc > 1:
                    with nc.allow_low_precision(reason="den partials"):
                        nc.vector.tensor_tensor(
                            out=st["dacc"][:, :, 0:w],
                            in0=st["dacc"][:, :, 0:w],
                            in1=st["exp_t"][:, kc, :, 0:w],
                            op=mybir.AluOpType.add,
                        )

            def av_pair(s, kc):
                """attn@v, both heads column-tiled into one concurrent pass."""
                _, w = CHUNKS[s]
                st = state[s]
                if st["po"] is None:
                    st["po"] = op.tile([P, RC_W], f32, tag="po", name=f"po{s}")
                for h in range(HL):
                    nc.tensor.matmul(
                        st["po"][h * DH:(h + 1) * DH, 0:w],
                        v_sb[:, kc, h, :],
                        st["exp_t"][:, kc, h, 0:w],
                        start=(kc == 0), stop=(kc == RT - 1),
                    )

            def norm_chain(s):
                """Partition-sum+broadcast denominators (PE), recip, norm, AG."""
                r0, w = CHUNKS[s]
                st = state[s]
                pr = prp.tile([P, RC_W], f32, tag="pr", name=f"pr{s}")
                for h in range(HL):
                    nc.tensor.matmul(
                        pr[h * DH:(h + 1) * DH, 0:w],
                        ones_row[:, 0:DH],
                        st["dacc"][:, h, 0:w],
                        start=True, stop=True,
                    )
                rb = rsump.tile([P, RC_W], f32, tag="rb", name=f"rb{s}")
                nc.vector.reciprocal(out=rb[:, 0:w], in_=pr[:, 0:w])
                with nc.allow_low_precision(reason="attn bf16 wire"):
                    nc.vector.tensor_tensor(
                        out=attn_full[:, r0:r0 + w],
                        in0=st["po"][:, 0:w],
                        in1=rb[:, 0:w],
                        op=mybir.AluOpType.mult,
                    )
                nc.sync.dma_start(
                    out=ag_in[s][:, :], in_=attn_full[:, r0:r0 + w]
                )
                nc.gpsimd.collective_compute(
                    "AllGather",
                    mybir.AluOpType.bypass,
                    replica_groups=groups,
                    ins=[ag_in[s][:].opt()],
                    outs=[ag_out[s][:].opt()],
                )

            def proj_dma(s):
                """Gathered-heads load: one strided DMA on the SP queue.
                Its AG-wait head-blocks later SP DMAs, but those only feed
                the NEXT AllGather, which is serialized behind this one on
                the CC stream anyway."""
                _, w = CHUNKS[s]
                st = state[s]
                agt = agp.tile([P, DC, RC_W], dt_mm, tag="agt", name=f"agt{s}")
                st["agt"] = agt
                nc.sync.dma_start(
                    out=agt[:, :, 0:w],
                    in_=ag_out[s][:, :].rearrange("(c p) q -> p c q", p=P),
                )

            def proj(s):
                r0, w = CHUNKS[s]
                st = state[s]
                agt = st["agt"]
                pf = dump.tile([P, RC_W], f32, tag="warm", name=f"pf{s}")
                for kc in range(DC):
                    nc.tensor.matmul(
                        pf[:, 0:w], wo_sb[:, kc, :], agt[:, kc, 0:w],
                        start=(kc == 0), stop=False,
                    )
                nc.tensor.matmul(
                    pf[:, 0:w], bo_row, ones_row[0:1, 0:w],
                    start=False, stop=True,
                )
                ot = agp.tile([P, RC_W], f32, tag="ot", name=f"ot{s}")
                nc.vector.tensor_copy(out=ot[:, 0:w], in_=pf[:, 0:w])
                nc.sync.dma_start(out=out_d[:, r0:r0 + w], in_=ot[:, 0:w])

            for s in range(S):
                state[s] = {
                    "exp_t": expp.tile([P, RT, HL, RC_W], dt_mm, tag="exp",
                                       name=f"exp{s}"),
                    "dacc": rsump.tile([P, HL, RC_W], dt_mm, tag="dacc",
                                       name=f"dacc{s}"),
                    "po": None, "agt": None,
                }

            # deferred Q/V projections slotted into stage D's PE slack
            extra = {
                0: lambda: qkv_mm(prp, "pr", wq_sb, 0, qT, 1, True),
                2: lambda: qkv_mm(prp, "pr", wv_sb, 64, vT, 1, True),
                4: lambda: vt_batch(prp, "pr", 1),
                6: lambda: qkv_mm(prp, "pr", wv_sb, 64, vT, 2, True),
                8: lambda: vt_batch(prp, "pr", 2),
                10: lambda: qkv_mm(prp, "pr", wv_sb, 64, vT, 3, True),
                12: lambda: vt_batch(prp, "pr", 3),
                18: lambda: qkv_mm(prp, "pr", wq_sb, 0, qT, 2, True),
                34: lambda: qkv_mm(prp, "pr", wq_sb, 0, qT, 3, True),
            }

            total = S * RT + LAG
            for t in range(total):
                if t in extra:
                    extra[t]()
                s, kc = divmod(t, RT)
                if t < S * RT:
                    sim_group(s, kc)
                u = t - LAG
                if u >= 0:
                    us, ukc = divmod(u, RT)
                    av_pair(us, ukc)
                    if ukc == RT - 1:
                        norm_chain(us)
            # all projections in the drain: the PE stream never touches
            # AG-dependent work, so collectives can never stall compute
            for s in range(S):
                proj_dma(s)
                proj(s)

            if DEBUG:
                dbg_sb = agp.tile([P, N + 128], f32, tag="dbg")
                for src, dst in ((qT, dbg_qt), (kT, dbg_kt)):
                    nc.vector.tensor_copy(out=dbg_sb[:, 0:N], in_=src)
                    nc.sync.dma_start(out=dst[:, :], in_=dbg_sb[:, 0:N])
                nc.vector.tensor_copy(
                    out=dbg_sb[:, 0:RT * HL * DH],
                    in_=v_sb[:].rearrange("p a b c -> p (a b c)"),
                )
                nc.sync.dma_start(out=dbg_v[:, :],
                                  in_=dbg_sb[:, 0:RT * HL * DH])
                nc.vector.tensor_copy(out=dbg_sb[0:DH, 0:N],
                                      in_=attn_full[0:DH, :])
                nc.sync.dma_start(out=dbg_attn[:, :], in_=dbg_sb[0:DH, 0:N])

    if not nc.is_finalized():
        nc.finalize()
    return nc


def _get_built():
    global _BUILT
    if _BUILT is None:
        _BUILT = _build()
    return _BUILT


def _shard_inputs(x, ln_scale, ln_bias, w_qkv, w_out, b_out):
    """Host-side sharding: slice per-head weight columns, fold LN params."""
    x = np.ascontiguousarray(np.asarray(x, np.float32))
    ln_scale = np.asarray(ln_scale, np.float32)
    ln_bias = np.asarray(ln_bias, np.float32)
    w_qkv = np.asarray(w_qkv, np.float32)
    w_out = np.asarray(w_out, np.float32)
    b_out = np.asarray(b_out, np.float32)

    w_np = {"f32": np.float32, "f32r": np.float32,
            "bf16": ml_dtypes.bfloat16}[MM_DT]

    in_maps = []
    for ci in range(NCORES):
        c0 = ci * HC
        sl = {}
        for name, off in (("q", 0), ("k", HEADS * DH), ("v", 2 * HEADS * DH)):
            w = w_qkv[:, off + c0: off + c0 + HC]
            sl["w" + name] = np.ascontiguousarray(
                (ln_scale[:, None] * w).astype(w_np)
            )
            sl[name + "b"] = np.ascontiguousarray((ln_bias @ w).astype(w_np))
        sl["wo"] = np.ascontiguousarray(w_out[:, c0:c0 + HC].astype(w_np))
        sl["bo"] = np.ascontiguousarray(b_out[c0:c0 + HC])
        sl["x"] = x
        in_maps.append(sl)
    return in_maps


def kernel(x, ln_scale, ln_bias, w_qkv, w_out, b_out):
    from concourse.bass_utils import run_bass_kernel_spmd

    nc = _get_built()
    in_maps = _shard_inputs(x, ln_scale, ln_bias, w_qkv, w_out, b_out)
    res = run_bass_kernel_spmd(nc, in_maps, core_ids=list(range(NCORES)))
    shards = [res.results[ci]["out"] for ci in range(NCORES)]  # [128, 2048] each
    outT = np.concatenate(shards, axis=0)  # [1024, 2048]
    return np.ascontiguousarray(outT.T)
